# revision 9
# baseline (speedup 1.0000x reference)
"""Trainium2 Bass kernel for batched 64-point DCT (flattened-patch GEMM).

Reference computation: out = x.reshape(b, -1, 64) @ K, reshaped back.
Pure data parallel over 8 NeuronCores: core i handles batch i as a
[49152, 64] x [64, 64] GEMM.

v2 design (fp16, K-stationary):
  * All HBM I/O is fp16 -- the correctness gate is rel_err < 2e-2 and
    fp16 rounding contributes ~1e-3, so this halves the DMA traffic
    (12 MB -> 6 MB in + 6 MB out per core), which is the roofline.
  * Device input layout  xth[(z*64+s), pair] = x[2*pair+z, s]: a
    [128, 24576] fp16 matrix (host-prepared).  128 partitions keeps all
    16 SDMA engines busy.
  * The DCT basis is the STATIONARY operand: blockdiag(K, K) [128, 128]
    is loaded into the PE array once; x streams through as the moving
    operand, 512 columns per matmul (one PSUM bank per matmul):
      out[(z*64+f), pair] = sum_s K[s, f] * x[2*pair+z, s]
  * PSUM [128, 512] fp32 -> SBUF fp16 copies alternate DVE/ACT; the
    fp16 [128, 24576] result DMAs out contiguously and the host
    un-transposes ((z,f),pair -> patch-major) while upcasting to fp32.
"""

import numpy as np

import concourse.mybir as mybir
from concourse import bacc
from concourse.bass_utils import run_bass_kernel_spmd
from concourse.tile import TileContext

P = 128    # SBUF partitions
S = 64     # DCT size (contraction dim)
NMM = 512  # moving columns per matmul = one PSUM bank of fp32
MMS_PER_TILE = 4
PAIRS_PER_TILE = NMM * MMS_PER_TILE     # 2048 pair-columns per macro-tile
N_CORES = 8
FP16 = mybir.dt.float16


def in_tile_schedule(n_pairs: int) -> list[int]:
    """Pair-columns per input DMA: a small head tile (goes out on the
    gpsimd SWDGE queue, whose descriptor generation runs in parallel with
    the HWDGE path, so the HBM stream starts ~1.5 us earlier), 1 MB loads
    in the steady state (fewer DMA boundaries = fewer queue stalls), and
    512 KB tails so the final matmuls aren't gated on a full 1 MB."""
    assert n_pairs % 4096 == 0
    return [2048] + [4096] * (n_pairs // 4096 - 1) + [2048]


def build_kernel(n_patches: int):
    n_pairs = n_patches // 2
    in_sched = in_tile_schedule(n_pairs)
    n_ctiles = n_pairs // PAIRS_PER_TILE
    nc = bacc.Bacc(
        "TRN2",
        target_bir_lowering=False,
        debug=False,
        enable_asserts=False,
        num_devices=N_CORES,
    )
    # xth[(z*64+s), pair] = x[2*pair+z, s], prepared host-side (fp16).
    x = nc.dram_tensor("x", [P, n_pairs], FP16, kind="ExternalInput")
    # host-prepared blockdiag(K, K), fp16
    k = nc.dram_tensor("k", [P, P], FP16, kind="ExternalInput")
    # yth[(z*64+f), pair] = y[2*pair+z, f] -- host un-transposes.
    y = nc.dram_tensor("y", [P, n_pairs], FP16, kind="ExternalOutput")

    xf = x.ap()
    yf = y.ap()

    with TileContext(nc) as tc:
        with (
            tc.tile_pool(name="consts", bufs=1) as consts,
            tc.tile_pool(name="xin", bufs=1) as x_pool,
            tc.tile_pool(name="outsb", bufs=1) as out_pool,
            tc.tile_pool(name="pout", bufs=8, space="PSUM") as pout_pool,
        ):
            kblk = consts.tile([P, P], FP16)
            # kblk heads the Sync queue: it's tiny (32 KB) and the first
            # matmul is gated on it.
            nc.sync.dma_start(out=kblk[:], in_=k.ap())

            # The whole 6 MB input fits in SBUF: every input tile gets its
            # own buffer, so all loads are issued back-to-back with no
            # buffer-recycle stalls.
            in_tiles = []   # (tile, first_pair, npair)
            col = 0
            for ti, npair in enumerate(in_sched):
                x_tile = x_pool.tile([P, npair], FP16, name=f"x{ti}")
                eng = nc.gpsimd if ti == 0 else nc.sync
                eng.dma_start(out=x_tile[:], in_=xf[:, col : col + npair])
                in_tiles.append((x_tile, col, npair))
                col += npair

            def x_slice(first_pair: int, width: int):
                for t, c0, npair in in_tiles:
                    if c0 <= first_pair and first_pair + width <= c0 + npair:
                        return t[:, first_pair - c0 : first_pair - c0 + width]
                raise AssertionError("unaligned compute tile")

            mm_i = 0
            for ci in range(n_ctiles):
                base = ci * PAIRS_PER_TILE
                out_sb = out_pool.tile(
                    [P, PAIRS_PER_TILE], FP16, name=f"o{ci}"
                )
                for m in range(MMS_PER_TILE):
                    po = pout_pool.tile([P, NMM], mybir.dt.float32)
                    nc.tensor.matmul(
                        po[:],
                        lhsT=kblk[:],
                        rhs=x_slice(base + NMM * m, NMM),
                        start=True,
                        stop=True,
                    )
                    dst = out_sb[:, NMM * m : NMM * (m + 1)]
                    if mm_i % 2 == 0:
                        nc.vector.tensor_copy(dst, po[:])
                    else:
                        nc.scalar.copy(dst, po[:])
                    mm_i += 1
                # store on the Scalar hwdge queue so it overlaps the
                # Sync-queue input stream
                nc.scalar.dma_start(
                    out=yf[:, base : base + PAIRS_PER_TILE], in_=out_sb[:]
                )
    nc.compile()
    return nc


def shard_input(x_core: np.ndarray) -> np.ndarray:
    """[n_patches, 64] fp32 -> [128, n_pairs] fp16 device layout.

    Row r = z*64 + s, column p: holds x[2p+z, s].
    """
    n = x_core.shape[0]
    x16 = x_core.astype(np.float16)
    # [n/2, 2, 64] -> [2, 64, n/2] -> [128, n/2]
    return np.ascontiguousarray(
        x16.reshape(n // 2, 2, S).transpose(1, 2, 0).reshape(P, n // 2)
    )


def unshard_output(y_dev: np.ndarray, n_patches: int) -> np.ndarray:
    """[128, n_pairs] fp16 device layout -> [n_patches, 64] fp32."""
    return (
        y_dev.reshape(2, S, n_patches // 2)
        .transpose(2, 0, 1)
        .reshape(n_patches, S)
        .astype(np.float32)
    )


def make_kblk(kmat: np.ndarray) -> np.ndarray:
    kblk_host = np.zeros((P, P), dtype=np.float16)
    kblk_host[:S, :S] = kmat
    kblk_host[S:, S:] = kmat
    return kblk_host


def _run(x_full: np.ndarray, kmat: np.ndarray, **spmd_kwargs):
    b, c, h, w = x_full.shape
    assert b == N_CORES, f"expected batch {N_CORES}, got {b}"
    n_patches = c * h * w // S
    nc = build_kernel(n_patches)
    kblk_host = make_kblk(kmat)
    in_maps = [
        {"x": shard_input(x_full[i].reshape(n_patches, S)), "k": kblk_host}
        for i in range(b)
    ]
    res = run_bass_kernel_spmd(
        nc, in_maps, core_ids=list(range(N_CORES)), **spmd_kwargs
    )
    out = np.stack(
        [
            unshard_output(res.results[i]["y"], n_patches).reshape(c, h, w)
            for i in range(b)
        ],
        axis=0,
    )
    return out, res


def kernel(inputs, kernel):
    x_full = np.ascontiguousarray(np.asarray(inputs, dtype=np.float32))
    kmat = np.asarray(kernel, dtype=np.float32)
    out, _ = _run(x_full, kmat)
    return out


# revision 11
# speedup vs baseline: 1.0150x; 1.0150x over previous
"""Trainium2 Bass kernel for batched 64-point DCT (flattened-patch GEMM).

Reference computation: out = x.reshape(b, -1, 64) @ K, reshaped back.
Pure data parallel over 8 NeuronCores: core i handles batch i as a
[49152, 64] x [64, 64] GEMM.

v2 design (fp16, K-stationary):
  * All HBM I/O is fp16 -- the correctness gate is rel_err < 2e-2 and
    fp16 rounding contributes ~1e-3, so this halves the DMA traffic
    (12 MB -> 6 MB in + 6 MB out per core), which is the roofline.
  * Device input layout  xth[(z*64+s), pair] = x[2*pair+z, s]: a
    [128, 24576] fp16 matrix (host-prepared).  128 partitions keeps all
    16 SDMA engines busy.
  * The DCT basis is the STATIONARY operand: blockdiag(K, K) [128, 128]
    is loaded into the PE array once; x streams through as the moving
    operand, 512 columns per matmul (one PSUM bank per matmul):
      out[(z*64+f), pair] = sum_s K[s, f] * x[2*pair+z, s]
  * PSUM [128, 512] fp32 -> SBUF fp16 copies alternate DVE/ACT; the
    fp16 [128, 24576] result DMAs out contiguously and the host
    un-transposes ((z,f),pair -> patch-major) while upcasting to fp32.
"""

import numpy as np

import concourse.mybir as mybir
from concourse import bacc
from concourse.bass_utils import run_bass_kernel_spmd
from concourse.tile import TileContext

P = 128    # SBUF partitions
S = 64     # DCT size (contraction dim)
NMM = 512  # moving columns per matmul = one PSUM bank of fp32
MMS_PER_TILE = 4
PAIRS_PER_TILE = NMM * MMS_PER_TILE     # 2048 pair-columns per macro-tile
N_CORES = 8
FP16 = mybir.dt.float16


def in_tile_schedule(n_pairs: int) -> list[int]:
    """Pair-columns per input DMA: 1 MB loads in the steady state (fewer
    DMA boundaries = fewer queue stalls), 512 KB head/tail so the first
    matmul starts sooner and the final matmuls aren't gated on a full
    1 MB landing."""
    assert n_pairs % 4096 == 0
    return [2048] + [4096] * (n_pairs // 4096 - 1) + [2048]


def build_kernel(n_patches: int):
    n_pairs = n_patches // 2
    in_sched = in_tile_schedule(n_pairs)
    n_ctiles = n_pairs // PAIRS_PER_TILE
    nc = bacc.Bacc(
        "TRN2",
        target_bir_lowering=False,
        debug=False,
        enable_asserts=False,
        num_devices=N_CORES,
    )
    # xth[(z*64+s), pair] = x[2*pair+z, s], prepared host-side (fp16).
    x = nc.dram_tensor("x", [P, n_pairs], FP16, kind="ExternalInput")
    # host-prepared blockdiag(K, K), fp16
    k = nc.dram_tensor("k", [P, P], FP16, kind="ExternalInput")
    # yth[(z*64+f), pair] = y[2*pair+z, f] -- host un-transposes.
    y = nc.dram_tensor("y", [P, n_pairs], FP16, kind="ExternalOutput")

    xf = x.ap()
    yf = y.ap()

    with TileContext(nc) as tc:
        with (
            tc.tile_pool(name="consts", bufs=1) as consts,
            tc.tile_pool(name="xin", bufs=1) as x_pool,
            tc.tile_pool(name="outsb", bufs=1) as out_pool,
            tc.tile_pool(name="pout", bufs=8, space="PSUM") as pout_pool,
        ):
            kblk = consts.tile([P, P], FP16)
            # kblk heads the Sync queue: it's tiny (32 KB) and the first
            # matmul is gated on it.
            nc.sync.dma_start(out=kblk[:], in_=k.ap())

            # The whole 6 MB input fits in SBUF: every input tile gets its
            # own buffer, so all loads are issued back-to-back with no
            # buffer-recycle stalls.
            in_tiles = []   # (tile, first_pair, npair)
            col = 0
            for ti, npair in enumerate(in_sched):
                x_tile = x_pool.tile([P, npair], FP16, name=f"x{ti}")
                nc.sync.dma_start(out=x_tile[:], in_=xf[:, col : col + npair])
                in_tiles.append((x_tile, col, npair))
                col += npair

            def x_slice(first_pair: int, width: int):
                for t, c0, npair in in_tiles:
                    if c0 <= first_pair and first_pair + width <= c0 + npair:
                        return t[:, first_pair - c0 : first_pair - c0 + width]
                raise AssertionError("unaligned compute tile")

            mm_i = 0
            for ci in range(n_ctiles):
                base = ci * PAIRS_PER_TILE
                out_sb = out_pool.tile(
                    [P, PAIRS_PER_TILE], FP16, name=f"o{ci}"
                )
                for m in range(MMS_PER_TILE):
                    po = pout_pool.tile([P, NMM], mybir.dt.float32)
                    nc.tensor.matmul(
                        po[:],
                        lhsT=kblk[:],
                        rhs=x_slice(base + NMM * m, NMM),
                        start=True,
                        stop=True,
                    )
                    dst = out_sb[:, NMM * m : NMM * (m + 1)]
                    if mm_i % 2 == 0:
                        nc.vector.tensor_copy(dst, po[:])
                    else:
                        nc.scalar.copy(dst, po[:])
                    mm_i += 1
                # store on the Scalar hwdge queue so it overlaps the
                # Sync-queue input stream
                nc.scalar.dma_start(
                    out=yf[:, base : base + PAIRS_PER_TILE], in_=out_sb[:]
                )
    nc.compile()
    return nc


def shard_input(x_core: np.ndarray) -> np.ndarray:
    """[n_patches, 64] fp32 -> [128, n_pairs] fp16 device layout.

    Row r = z*64 + s, column p: holds x[2p+z, s].
    """
    n = x_core.shape[0]
    x16 = x_core.astype(np.float16)
    # [n/2, 2, 64] -> [2, 64, n/2] -> [128, n/2]
    return np.ascontiguousarray(
        x16.reshape(n // 2, 2, S).transpose(1, 2, 0).reshape(P, n // 2)
    )


def unshard_output(y_dev: np.ndarray, n_patches: int) -> np.ndarray:
    """[128, n_pairs] fp16 device layout -> [n_patches, 64] fp32."""
    return (
        y_dev.reshape(2, S, n_patches // 2)
        .transpose(2, 0, 1)
        .reshape(n_patches, S)
        .astype(np.float32)
    )


def make_kblk(kmat: np.ndarray) -> np.ndarray:
    kblk_host = np.zeros((P, P), dtype=np.float16)
    kblk_host[:S, :S] = kmat
    kblk_host[S:, S:] = kmat
    return kblk_host


def _run(x_full: np.ndarray, kmat: np.ndarray, **spmd_kwargs):
    b, c, h, w = x_full.shape
    assert b == N_CORES, f"expected batch {N_CORES}, got {b}"
    n_patches = c * h * w // S
    nc = build_kernel(n_patches)
    kblk_host = make_kblk(kmat)
    in_maps = [
        {"x": shard_input(x_full[i].reshape(n_patches, S)), "k": kblk_host}
        for i in range(b)
    ]
    res = run_bass_kernel_spmd(
        nc, in_maps, core_ids=list(range(N_CORES)), **spmd_kwargs
    )
    out = np.stack(
        [
            unshard_output(res.results[i]["y"], n_patches).reshape(c, h, w)
            for i in range(b)
        ],
        axis=0,
    )
    return out, res


def kernel(inputs, kernel):
    x_full = np.ascontiguousarray(np.asarray(inputs, dtype=np.float32))
    kmat = np.asarray(kernel, dtype=np.float32)
    out, _ = _run(x_full, kmat)
    return out


# revision 14
# speedup vs baseline: 1.1251x; 1.1086x over previous
"""Trainium2 Bass kernel for batched 64-point DCT (flattened-patch GEMM).

Reference computation: out = x.reshape(b, -1, 64) @ K, reshaped back.
Pure data parallel over 8 NeuronCores: core i handles batch i as a
[49152, 64] x [64, 64] GEMM.

v2 design (fp16, K-stationary):
  * All HBM I/O is fp16 -- the correctness gate is rel_err < 2e-2 and
    fp16 rounding contributes ~1e-3, so this halves the DMA traffic
    (12 MB -> 6 MB in + 6 MB out per core), which is the roofline.
  * Device input layout  xth[(z*64+s), pair] = x[2*pair+z, s]: a
    [128, 24576] fp16 matrix (host-prepared).  128 partitions keeps all
    16 SDMA engines busy.
  * The DCT basis is the STATIONARY operand: blockdiag(K, K) [128, 128]
    is loaded into the PE array once; x streams through as the moving
    operand, 512 columns per matmul (one PSUM bank per matmul):
      out[(z*64+f), pair] = sum_s K[s, f] * x[2*pair+z, s]
  * PSUM [128, 512] fp32 -> SBUF fp16 copies alternate DVE/ACT; the
    fp16 [128, 24576] result DMAs out contiguously and the host
    un-transposes ((z,f),pair -> patch-major) while upcasting to fp32.
"""

import numpy as np

import concourse.mybir as mybir
from concourse import bacc
from concourse.bass_utils import run_bass_kernel_spmd
from concourse.tile import TileContext

P = 128    # SBUF partitions
S = 64     # DCT size (contraction dim)
NMM = 512  # moving columns per matmul = one PSUM bank of fp32
MMS_PER_TILE = 4
PAIRS_PER_TILE = NMM * MMS_PER_TILE     # 2048 pair-columns per macro-tile
N_CORES = 8
FP16 = mybir.dt.float16


def in_tile_schedule(n_pairs: int) -> list[int]:
    """Pair-columns per input DMA: 1 MB loads in the steady state (fewer
    DMA boundaries = fewer queue stalls), small head tiles so the first
    matmul/store start sooner, small tail tiles so the final
    matmul->copy->store->receipt drain after the last input byte is
    short."""
    assert n_pairs % 4096 == 0
    return [512, 1536] + [4096] * (n_pairs // 4096 - 1) + [1536, 512]


def compute_tile_schedule(n_pairs: int) -> list[int]:
    """Pair-columns per compute/output tile; each must lie inside one
    input tile."""
    assert n_pairs % PAIRS_PER_TILE == 0
    n_mid = n_pairs // PAIRS_PER_TILE - 2
    return [512, 1536] + [PAIRS_PER_TILE] * n_mid + [1536, 512]


def build_kernel(n_patches: int):
    n_pairs = n_patches // 2
    in_sched = in_tile_schedule(n_pairs)
    c_sched = compute_tile_schedule(n_pairs)
    nc = bacc.Bacc(
        "TRN2",
        target_bir_lowering=False,
        debug=False,
        enable_asserts=False,
        num_devices=N_CORES,
    )
    # xth[(z*64+s), pair] = x[2*pair+z, s], prepared host-side (fp16).
    x = nc.dram_tensor("x", [P, n_pairs], FP16, kind="ExternalInput")
    # host-prepared blockdiag(K, K), fp16
    k = nc.dram_tensor("k", [P, P], FP16, kind="ExternalInput")
    # yth[(z*64+f), pair] = y[2*pair+z, f] -- host un-transposes.
    y = nc.dram_tensor("y", [P, n_pairs], FP16, kind="ExternalOutput")

    xf = x.ap()
    yf = y.ap()

    with TileContext(nc) as tc:
        with (
            tc.tile_pool(name="consts", bufs=1) as consts,
            tc.tile_pool(name="xin", bufs=1) as x_pool,
            tc.tile_pool(name="outsb", bufs=1) as out_pool,
            tc.tile_pool(name="pout", bufs=8, space="PSUM") as pout_pool,
        ):
            kblk = consts.tile([P, P], FP16)
            # kblk heads the Sync queue: it's tiny (32 KB) and the first
            # matmul is gated on it.
            nc.sync.dma_start(out=kblk[:], in_=k.ap())

            # The whole 6 MB input fits in SBUF: every input tile gets its
            # own buffer, so all loads are issued back-to-back with no
            # buffer-recycle stalls.
            in_tiles = []   # (tile, first_pair, npair)
            col = 0
            for ti, npair in enumerate(in_sched):
                x_tile = x_pool.tile([P, npair], FP16, name=f"x{ti}")
                nc.sync.dma_start(out=x_tile[:], in_=xf[:, col : col + npair])
                in_tiles.append((x_tile, col, npair))
                col += npair

            def x_slice(first_pair: int, width: int):
                for t, c0, npair in in_tiles:
                    if c0 <= first_pair and first_pair + width <= c0 + npair:
                        return t[:, first_pair - c0 : first_pair - c0 + width]
                raise AssertionError("unaligned compute tile")

            mm_i = 0
            base = 0
            for ci, cw in enumerate(c_sched):
                out_sb = out_pool.tile([P, cw], FP16, name=f"o{ci}")
                nmm_full, rem = divmod(cw, NMM)
                widths = [NMM] * nmm_full + ([rem] if rem else [])
                moff = 0
                for w in widths:
                    po = pout_pool.tile([P, NMM], mybir.dt.float32)
                    nc.tensor.matmul(
                        po[:, :w],
                        lhsT=kblk[:],
                        rhs=x_slice(base + moff, w),
                        start=True,
                        stop=True,
                    )
                    dst = out_sb[:, moff : moff + w]
                    if mm_i % 2 == 0:
                        nc.vector.tensor_copy(dst, po[:, :w])
                    else:
                        nc.scalar.copy(dst, po[:, :w])
                    mm_i += 1
                    moff += w
                # store on the Scalar hwdge queue so it overlaps the
                # Sync-queue input stream
                nc.scalar.dma_start(out=yf[:, base : base + cw], in_=out_sb[:])
                base += cw
    nc.compile()
    return nc


def shard_input(x_core: np.ndarray) -> np.ndarray:
    """[n_patches, 64] fp32 -> [128, n_pairs] fp16 device layout.

    Row r = z*64 + s, column p: holds x[2p+z, s].
    """
    n = x_core.shape[0]
    x16 = x_core.astype(np.float16)
    # [n/2, 2, 64] -> [2, 64, n/2] -> [128, n/2]
    return np.ascontiguousarray(
        x16.reshape(n // 2, 2, S).transpose(1, 2, 0).reshape(P, n // 2)
    )


def unshard_output(y_dev: np.ndarray, n_patches: int) -> np.ndarray:
    """[128, n_pairs] fp16 device layout -> [n_patches, 64] fp32."""
    return (
        y_dev.reshape(2, S, n_patches // 2)
        .transpose(2, 0, 1)
        .reshape(n_patches, S)
        .astype(np.float32)
    )


def make_kblk(kmat: np.ndarray) -> np.ndarray:
    kblk_host = np.zeros((P, P), dtype=np.float16)
    kblk_host[:S, :S] = kmat
    kblk_host[S:, S:] = kmat
    return kblk_host


def _run(x_full: np.ndarray, kmat: np.ndarray, **spmd_kwargs):
    b, c, h, w = x_full.shape
    assert b == N_CORES, f"expected batch {N_CORES}, got {b}"
    n_patches = c * h * w // S
    nc = build_kernel(n_patches)
    kblk_host = make_kblk(kmat)
    in_maps = [
        {"x": shard_input(x_full[i].reshape(n_patches, S)), "k": kblk_host}
        for i in range(b)
    ]
    res = run_bass_kernel_spmd(
        nc, in_maps, core_ids=list(range(N_CORES)), **spmd_kwargs
    )
    out = np.stack(
        [
            unshard_output(res.results[i]["y"], n_patches).reshape(c, h, w)
            for i in range(b)
        ],
        axis=0,
    )
    return out, res


def kernel(inputs, kernel):
    x_full = np.ascontiguousarray(np.asarray(inputs, dtype=np.float32))
    kmat = np.asarray(kernel, dtype=np.float32)
    out, _ = _run(x_full, kmat)
    return out
